# revision 5
# baseline (speedup 1.0000x reference)
"""Causal self-attention (per-head full-D k/q, DH-wide v) on 8 trn2 cores.

Sharding: tensor-parallel over heads. Core c owns heads (2c, 2c+1).
Each core computes, for all 4 batches:
  y^T[h] = (x @ Wkqv[h] + bkqv[h])^T        (e on partitions, tokens free)
  S^T    = k @ q^T / 32                     (m on partitions, n free)
  P^T    = exp(S^T) * causal_mask           (unnormalized, bf16)
  O^T_aug= [v | 1]^T-stacked @ P^T          (row 64 = softmax denominator)
  O^T    = O^T[0:64] / den                  (per-column normalize)
  partial= [O^T(h0); O^T(h1)].T @ Wp[128c:128c+128]   (f32, DMA'd out)
Host sums the 8 partials and adds bp.

Layouts chosen so no on-device transposes of large tensors are needed:
x is pre-transposed on host; k^T/q^T come out of the projection directly;
P^T is what the A@v matmul wants as moving operand; only v needs an
on-device transpose (8 small 64x128 PE transposes per (b,h)).
"""

import numpy as np
import ml_dtypes

import concourse.bass as bass
import concourse.bacc as bacc
import concourse.tile as tile
from concourse import mybir
from concourse.bass_utils import run_bass_kernel_spmd

BF16 = mybir.dt.bfloat16
F32 = mybir.dt.float32
AF = mybir.ActivationFunctionType

B, N, D, H, DH = 4, 1024, 1024, 16, 64
E = 2 * D + DH          # 2112 = per-head kqv output width
P = 128
NCORES = 8
HL = H // NCORES        # 2 local heads per core
DT = D // P             # 8 contraction tiles
ET = 17                 # e tiles: 8 k + 8 q + 1 v(64 rows)
NB = N // 512           # 2 moving-dim blocks
NT = N // P             # 8 token tiles

_CACHE = {}


def _build_nc():
    nc = bacc.Bacc(
        "TRN2",
        target_bir_lowering=False,
        debug=False,
        enable_asserts=True,
        num_devices=NCORES,
    )
    xt_d = nc.declare_dram_parameter("xt", [B, D, N], BF16, isOutput=False)
    wk_d = nc.declare_dram_parameter("wkqv", [HL, D, E], BF16, isOutput=False)
    bias_d = nc.declare_dram_parameter("bias", [HL, P, ET], F32, isOutput=False)
    wp_d = nc.declare_dram_parameter("wp", [P, D], BF16, isOutput=False)
    mask_d = nc.declare_dram_parameter("masks", [4, P, 512], BF16, isOutput=False)
    id_d = nc.declare_dram_parameter("ident", [P, P], BF16, isOutput=False)
    ones_d = nc.declare_dram_parameter("ones", [1, DH], F32, isOutput=False)
    out_d = nc.declare_dram_parameter("out", [B, N, D], F32, isOutput=True)

    with tile.TileContext(nc) as tc:
        with (
            tc.tile_pool(name="const", bufs=1) as constp,
            tc.tile_pool(name="wpool", bufs=1) as wpool,
            tc.tile_pool(name="xpool", bufs=2) as xpool,
            tc.tile_pool(name="kqpool", bufs=1) as kqpool,
            tc.tile_pool(name="vpool", bufs=2) as vpool,
            tc.tile_pool(name="ptpool", bufs=4) as ptpool,
            tc.tile_pool(name="otpool", bufs=2) as otpool,
            tc.tile_pool(name="stpool", bufs=3) as stpool,
            tc.tile_pool(name="pspool", bufs=1, space="PSUM") as pspool,
        ):
            # ---- resident constants/weights ----
            wk_sb = wpool.tile([P, HL * DT * E], BF16, name="wk_sb")
            for h in range(HL):
                for d in range(DT):
                    nc.sync.dma_start(
                        out=wk_sb[:, (h * DT + d) * E:(h * DT + d + 1) * E],
                        in_=wk_d[h, d * P:(d + 1) * P, :],
                    )
            bias_sb = constp.tile([P, HL * ET], F32, name="bias_sb")
            for h in range(HL):
                nc.sync.dma_start(
                    out=bias_sb[:, h * ET:(h + 1) * ET], in_=bias_d[h]
                )
            wp_sb = constp.tile([P, D], BF16, name="wp_sb")
            nc.sync.dma_start(out=wp_sb[:], in_=wp_d[:])
            mask_sb = constp.tile([P, 4 * 512], BF16, name="mask_sb")
            for m in range(4):
                nc.sync.dma_start(
                    out=mask_sb[:, m * 512:(m + 1) * 512], in_=mask_d[m]
                )
            id_sb = constp.tile([P, P], BF16, name="id_sb")
            nc.sync.dma_start(out=id_sb[:], in_=id_d[:])
            ones_sb = constp.tile([1, DH], F32, name="ones_sb")
            nc.sync.dma_start(out=ones_sb[:], in_=ones_d[:])

            for b in range(B):
                xt_sb = xpool.tile([P, DT * N], BF16, tag="xt", name="xt_sb")
                for d in range(DT):
                    nc.sync.dma_start(
                        out=xt_sb[:, d * N:(d + 1) * N],
                        in_=xt_d[b, d * P:(d + 1) * P, :],
                    )
                ostack = otpool.tile([P, N], BF16, tag="ostack", name="ostack")

                for h in range(HL):
                    kt_sb = kqpool.tile([P, DT * N], BF16, tag="kt", name="kt_sb")
                    qt_sb = kqpool.tile([P, DT * N], BF16, tag="qt", name="qt_sb")
                    vt_sb = vpool.tile([DH, N], BF16, tag="vt", name="vt_sb")

                    # ---- kqv projection: y^T[e_tile, n] ----
                    for t in range(ET):
                        rows = P if t < 16 else DH
                        ecol = t * P    # t==16 -> 2048 (v block)
                        for j in range(NB):
                            ps_y = pspool.tile(
                                [P, 512], F32, tag="ps", bufs=4, name="ps_y"
                            )
                            for d in range(DT):
                                wofs = (h * DT + d) * E + ecol
                                nc.tensor.matmul(
                                    ps_y[:rows, :],
                                    lhsT=wk_sb[:, wofs:wofs + rows],
                                    rhs=xt_sb[:, d * N + j * 512: d * N + j * 512 + 512],
                                    start=(d == 0),
                                    stop=(d == DT - 1),
                                )
                            bias_ap = bias_sb[:rows, h * ET + t: h * ET + t + 1]
                            if t < 8:
                                dest = kt_sb[:, t * N + j * 512: t * N + j * 512 + 512]
                            elif t < 16:
                                dest = qt_sb[:, (t - 8) * N + j * 512: (t - 8) * N + j * 512 + 512]
                            else:
                                dest = vt_sb[:, j * 512:(j + 1) * 512]
                            nc.vector.tensor_scalar_add(dest, ps_y[:rows, :], bias_ap)

                    # ---- v: transpose to [m, dh] and append ones column ----
                    v_sb = vpool.tile([P, NT * (DH + 1)], BF16, tag="vaug", name="v_sb")
                    nc.vector.memset(v_sb[:, :], 1.0)
                    for i in range(NT):
                        ps_t = pspool.tile([P, DH], BF16, tag="ps", bufs=4, name="ps_t")
                        nc.tensor.transpose(
                            ps_t[:, :],
                            vt_sb[:, i * P:(i + 1) * P],
                            id_sb[:DH, :DH],
                        )
                        nc.scalar.activation(
                            v_sb[:, i * (DH + 1): i * (DH + 1) + DH], ps_t[:, :], AF.Copy
                        )

                    # ---- attention: S^T tiles, exp, mask, P^T @ [v|1] ----
                    for j in range(NB):
                        nm = 4 * (j + 1)     # causal: valid m tiles for this n block
                        ps_o = pspool.tile([DH + 1, 512], F32, tag="po", bufs=2, name="ps_o")
                        for i in range(nm):
                            ps_s = pspool.tile([P, 512], F32, tag="ps", bufs=4, name="ps_s")
                            for e in range(DT):
                                nc.tensor.matmul(
                                    ps_s[:],
                                    lhsT=kt_sb[:, e * N + i * P: e * N + i * P + P],
                                    rhs=qt_sb[:, e * N + j * 512: e * N + j * 512 + 512],
                                    start=(e == 0),
                                    stop=(e == DT - 1),
                                )
                            pt = ptpool.tile([P, 512], BF16, tag="pt", name="pt")
                            nc.scalar.activation(pt[:], ps_s[:], AF.Exp, scale=1.0 / 32.0)
                            mi = i - 4 * j
                            if mi >= 0:  # partial (diagonal-crossing) tile
                                pt2 = ptpool.tile([P, 512], BF16, tag="pt", name="pt2")
                                nc.vector.tensor_mul(
                                    pt2[:], pt[:], mask_sb[:, mi * 512:(mi + 1) * 512]
                                )
                                pt = pt2
                            nc.tensor.matmul(
                                ps_o[:],
                                lhsT=v_sb[:, i * (DH + 1):(i + 1) * (DH + 1)],
                                rhs=pt[:],
                                start=(i == 0),
                                stop=(i == nm - 1),
                            )
                        # ---- normalize by denominator (row 64 of ps_o) ----
                        ot = otpool.tile([DH, 512], F32, tag="ot", name="ot")
                        nc.scalar.activation(ot[:], ps_o[:DH, :], AF.Copy)
                        den_row = otpool.tile([1, 512], F32, tag="den", name="den_row")
                        nc.scalar.activation(den_row[:], ps_o[DH:DH + 1, :], AF.Copy)
                        ps_d = pspool.tile([DH, 512], F32, tag="ps", bufs=4, name="ps_d")
                        nc.tensor.matmul(
                            ps_d[:], lhsT=ones_sb[:, :], rhs=den_row[:],
                            start=True, stop=True,
                        )
                        recip = stpool.tile([DH, 512], F32, tag="recip", name="recip")
                        nc.vector.reciprocal(recip[:], ps_d[:])
                        nc.vector.tensor_mul(
                            ostack[h * DH:(h + 1) * DH, j * 512:(j + 1) * 512],
                            ot[:], recip[:],
                        )

                # ---- output projection: both heads at once (K=128) ----
                for t in range(NT):
                    for j2 in range(NB):
                        ps_f = pspool.tile([P, 512], F32, tag="ps", bufs=4, name="ps_f")
                        nc.tensor.matmul(
                            ps_f[:],
                            lhsT=ostack[:, t * P:(t + 1) * P],
                            rhs=wp_sb[:, j2 * 512:(j2 + 1) * 512],
                            start=True, stop=True,
                        )
                        stage = stpool.tile([P, 512], F32, tag="stage", name="stage")
                        nc.scalar.activation(stage[:], ps_f[:], AF.Copy)
                        nc.sync.dma_start(
                            out=out_d[b, t * P:(t + 1) * P, j2 * 512:(j2 + 1) * 512],
                            in_=stage[:],
                        )
    nc.finalize()
    return nc


def _get_nc():
    if "nc" not in _CACHE:
        _CACHE["nc"] = _build_nc()
    return _CACHE["nc"]


def make_in_maps(x, Wkqv, bkqv, Wp):
    bf16 = ml_dtypes.bfloat16
    xt = np.ascontiguousarray(np.transpose(x, (0, 2, 1))).astype(bf16)
    pidx = np.arange(P)[:, None]
    fidx = np.arange(512)[None, :]
    masks = np.stack(
        [(pidx + P * i <= fidx) for i in range(4)]
    ).astype(bf16)
    ident = np.eye(P, dtype=bf16)
    ones = np.ones((1, DH), np.float32)
    in_maps = []
    for c in range(NCORES):
        wk = np.ascontiguousarray(Wkqv[HL * c:HL * (c + 1)]).astype(bf16)
        bk = np.asarray(bkqv[HL * c:HL * (c + 1)], np.float32)
        bias = np.zeros((HL, P, ET), np.float32)
        for t in range(16):
            bias[:, :, t] = bk[:, t * P:(t + 1) * P]
        bias[:, :DH, 16] = bk[:, 2 * D:]
        wp = np.ascontiguousarray(Wp[P * c:P * (c + 1)]).astype(bf16)
        in_maps.append({
            "xt": xt, "wkqv": wk, "bias": bias, "wp": wp,
            "masks": masks, "ident": ident, "ones": ones,
        })
    return in_maps


def run(x, Wkqv, bkqv, Wp, bp, trace=False):
    nc = _get_nc()
    in_maps = make_in_maps(x, Wkqv, bkqv, Wp)
    res = run_bass_kernel_spmd(nc, in_maps, core_ids=list(range(NCORES)), trace=trace)
    total = None
    for r in res.results:
        part = r["out"].astype(np.float64)
        total = part if total is None else total + part
    out = (total + np.asarray(bp, np.float64)).astype(np.float32)
    return out, res


def kernel(x, Wkqv, bkqv, Wp, bp):
    out, _ = run(x, Wkqv, bkqv, Wp, bp, trace=False)
    return out


# revision 9
# speedup vs baseline: 1.0581x; 1.0581x over previous
"""Causal self-attention (per-head full-D k/q, DH-wide v) on 8 trn2 cores.

Sharding: tensor-parallel over heads. Core c owns heads (2c, 2c+1).
Each core computes, for all 4 batches:
  y^T[h] = (x @ Wkqv[h] + bkqv[h])^T        (e on partitions, tokens free)
  S^T    = k @ q^T / 32                     (m on partitions, n free)
  P^T    = exp(S^T) * causal_mask           (unnormalized, bf16)
  O^T_aug= [v | 1]^T-stacked @ P^T          (row 64 = softmax denominator)
  O^T    = O^T[0:64] / den                  (per-column normalize)
  partial= [O^T(h0); O^T(h1)].T @ Wp[128c:128c+128]   (f32, DMA'd out)
Host sums the 8 partials and adds bp.

Layouts chosen so no on-device transposes of large tensors are needed:
x is pre-transposed on host; k^T/q^T come out of the projection directly;
P^T is what the A@v matmul wants as moving operand; only v needs an
on-device transpose (8 small 64x128 PE transposes per (b,h)).
"""

import numpy as np
import ml_dtypes

import concourse.bass as bass
import concourse.bacc as bacc
import concourse.tile as tile
from concourse import mybir
from concourse.bass_utils import run_bass_kernel_spmd

BF16 = mybir.dt.bfloat16
F32 = mybir.dt.float32
F32R = mybir.dt.float32r
AF = mybir.ActivationFunctionType

B, N, D, H, DH = 4, 1024, 1024, 16, 64
E = 2 * D + DH          # 2112 = per-head kqv output width
P = 128
NCORES = 8
HL = H // NCORES        # 2 local heads per core
DT = D // P             # 8 contraction tiles
ET = 17                 # e tiles: 8 k + 8 q + 1 v(64 rows)
NB = N // 512           # 2 moving-dim blocks
NT = N // P             # 8 token tiles

_CACHE = {}


def _build_nc():
    nc = bacc.Bacc(
        "TRN2",
        target_bir_lowering=False,
        debug=False,
        enable_asserts=True,
        num_devices=NCORES,
    )
    xt_d = nc.declare_dram_parameter("xt", [B, D, N], BF16, isOutput=False)
    wk_d = nc.declare_dram_parameter("wkqv", [HL, D, E], BF16, isOutput=False)
    bias_d = nc.declare_dram_parameter("bias", [HL, P, ET], F32, isOutput=False)
    wp_d = nc.declare_dram_parameter("wp", [P, D], BF16, isOutput=False)
    mask_d = nc.declare_dram_parameter("masks", [4, P, 512], BF16, isOutput=False)
    id_d = nc.declare_dram_parameter("ident", [P, P], BF16, isOutput=False)
    ones_d = nc.declare_dram_parameter("ones", [1, DH], F32R, isOutput=False)
    out_d = nc.declare_dram_parameter("out", [B, N, D], F32, isOutput=True)

    with tile.TileContext(nc) as tc:
        with (
            tc.tile_pool(name="const", bufs=1) as constp,
            tc.tile_pool(name="wpool", bufs=1) as wpool,
            tc.tile_pool(name="xpool", bufs=2) as xpool,
            tc.tile_pool(name="kqpool", bufs=1) as kqpool,
            tc.tile_pool(name="vpool", bufs=2) as vpool,
            tc.tile_pool(name="ptpool", bufs=4) as ptpool,
            tc.tile_pool(name="otpool", bufs=2) as otpool,
            tc.tile_pool(name="stpool", bufs=3) as stpool,
            tc.tile_pool(name="pspool", bufs=1, space="PSUM") as pspool,
        ):
            # ---- resident constants/weights ----
            # DMA issue order matters: the first projection matmuls need
            # wk[h0] + xt[b0] + bias, so those go first; everything else
            # (masks, identity, second head, Wp) is needed later and queues
            # behind them.
            wk_sb = wpool.tile([P, HL * DT * E], BF16, name="wk_sb")
            bias_sb = constp.tile([P, HL * ET], F32, name="bias_sb")
            wp_sb = constp.tile([P, D], BF16, name="wp_sb")
            mask_sb = constp.tile([P, 4 * 512], BF16, name="mask_sb")
            id_sb = constp.tile([P, P], BF16, name="id_sb")
            ones_sb = constp.tile([1, DH], F32R, name="ones_sb")

            def dma_wk(h, d):
                nc.sync.dma_start(
                    out=wk_sb[:, (h * DT + d) * E:(h * DT + d + 1) * E],
                    in_=wk_d[h, d * P:(d + 1) * P, :],
                )

            dma_wk(0, 0)
            for h in range(HL):
                nc.sync.dma_start(
                    out=bias_sb[:, h * ET:(h + 1) * ET], in_=bias_d[h]
                )

            for b in range(B):
                xt_sb = xpool.tile([P, DT * N], BF16, tag="xt", name="xt_sb")
                for d in range(DT):
                    nc.sync.dma_start(
                        out=xt_sb[:, d * N:(d + 1) * N],
                        in_=xt_d[b, d * P:(d + 1) * P, :],
                    )
                    if b == 0 and d >= 1:
                        dma_wk(0, d)
                if b == 0:
                    nc.sync.dma_start(out=id_sb[:], in_=id_d[:])
                    nc.sync.dma_start(out=ones_sb[:], in_=ones_d[:])
                    for m in range(4):
                        nc.sync.dma_start(
                            out=mask_sb[:, m * 512:(m + 1) * 512], in_=mask_d[m]
                        )
                    for d in range(DT):
                        dma_wk(1, d)
                    nc.sync.dma_start(out=wp_sb[:], in_=wp_d[:])
                ostack = otpool.tile([P, N], BF16, tag="ostack", name="ostack")

                for h in range(HL):
                    kt_sb = kqpool.tile([P, DT * N], BF16, tag="kt", name="kt_sb")
                    qt_sb = kqpool.tile([P, DT * N], BF16, tag="qt", name="qt_sb")
                    vt_sb = vpool.tile([DH, N], BF16, tag="vt", name="vt_sb")

                    # ---- kqv projection: y^T[e_tile, n] ----
                    for t in range(ET):
                        rows = P if t < 16 else DH
                        ecol = t * P    # t==16 -> 2048 (v block)
                        for j in range(NB):
                            ps_y = pspool.tile(
                                [P, 512], F32, tag="ps", bufs=3, name="ps_y"
                            )
                            for d in range(DT):
                                wofs = (h * DT + d) * E + ecol
                                nc.tensor.matmul(
                                    ps_y[:rows, :],
                                    lhsT=wk_sb[:, wofs:wofs + rows],
                                    rhs=xt_sb[:, d * N + j * 512: d * N + j * 512 + 512],
                                    start=(d == 0),
                                    stop=(d == DT - 1),
                                )
                            bias_ap = bias_sb[:rows, h * ET + t: h * ET + t + 1]
                            if t < 8:
                                dest = kt_sb[:, t * N + j * 512: t * N + j * 512 + 512]
                            elif t < 16:
                                dest = qt_sb[:, (t - 8) * N + j * 512: (t - 8) * N + j * 512 + 512]
                            else:
                                dest = vt_sb[:, j * 512:(j + 1) * 512]
                            nc.vector.tensor_scalar_add(dest, ps_y[:rows, :], bias_ap)

                    # ---- v: transpose to [m, dh] and append ones column ----
                    v_sb = vpool.tile([P, NT * (DH + 1)], BF16, tag="vaug", name="v_sb")
                    nc.vector.memset(v_sb[:, :], 1.0)
                    for i in range(NT):
                        ps_t = pspool.tile([P, DH], BF16, tag="ps", bufs=3, name="ps_t")
                        nc.tensor.transpose(
                            ps_t[:, :],
                            vt_sb[:, i * P:(i + 1) * P],
                            id_sb[:DH, :DH],
                        )
                        nc.scalar.activation(
                            v_sb[:, i * (DH + 1): i * (DH + 1) + DH], ps_t[:, :], AF.Copy
                        )

                    # ---- attention: S^T tiles, exp, mask, P^T @ [v|1] ----
                    for j in range(NB):
                        nm = 4 * (j + 1)     # causal: valid m tiles for this n block
                        ps_o = pspool.tile([DH + 1, 512], F32, tag="po", bufs=2, name="ps_o")
                        for i in range(nm):
                            ps_s = pspool.tile([P, 512], F32, tag="ps", bufs=3, name="ps_s")
                            for e in range(DT):
                                nc.tensor.matmul(
                                    ps_s[:],
                                    lhsT=kt_sb[:, e * N + i * P: e * N + i * P + P],
                                    rhs=qt_sb[:, e * N + j * 512: e * N + j * 512 + 512],
                                    start=(e == 0),
                                    stop=(e == DT - 1),
                                )
                            pt = ptpool.tile([P, 512], BF16, tag="pt", name="pt")
                            nc.scalar.activation(pt[:], ps_s[:], AF.Exp, scale=1.0 / 32.0)
                            mi = i - 4 * j
                            if mi >= 0:  # partial (diagonal-crossing) tile
                                pt2 = ptpool.tile([P, 512], BF16, tag="pt", name="pt2")
                                nc.vector.tensor_mul(
                                    pt2[:], pt[:], mask_sb[:, mi * 512:(mi + 1) * 512]
                                )
                                pt = pt2
                            nc.tensor.matmul(
                                ps_o[:],
                                lhsT=v_sb[:, i * (DH + 1):(i + 1) * (DH + 1)],
                                rhs=pt[:],
                                start=(i == 0),
                                stop=(i == nm - 1),
                            )
                        # ---- normalize by denominator (row 64 of ps_o) ----
                        ot = otpool.tile([DH, 512], F32, tag="ot", name="ot")
                        nc.scalar.activation(ot[:], ps_o[:DH, :], AF.Copy)
                        den_row = otpool.tile([1, 512], F32R, tag="den", name="den_row")
                        nc.scalar.activation(den_row[:], ps_o[DH:DH + 1, :], AF.Copy)
                        ps_d = pspool.tile([DH, 512], F32, tag="ps", bufs=3, name="ps_d")
                        nc.tensor.matmul(
                            ps_d[:],
                            lhsT=ones_sb[:, :],
                            rhs=den_row[:],
                            start=True, stop=True,
                        )
                        recip = stpool.tile([DH, 512], F32, tag="recip", name="recip")
                        nc.vector.reciprocal(recip[:], ps_d[:])
                        nc.vector.tensor_mul(
                            ostack[h * DH:(h + 1) * DH, j * 512:(j + 1) * 512],
                            ot[:], recip[:],
                        )

                # ---- output projection: both heads at once (K=128) ----
                for t in range(NT):
                    for j2 in range(NB):
                        ps_f = pspool.tile([P, 512], F32, tag="psf", bufs=2, name="ps_f")
                        nc.tensor.matmul(
                            ps_f[:],
                            lhsT=ostack[:, t * P:(t + 1) * P],
                            rhs=wp_sb[:, j2 * 512:(j2 + 1) * 512],
                            start=True, stop=True,
                        )
                        stage = stpool.tile([P, 512], F32, tag="stage", name="stage")
                        nc.scalar.activation(stage[:], ps_f[:], AF.Copy)
                        nc.sync.dma_start(
                            out=out_d[b, t * P:(t + 1) * P, j2 * 512:(j2 + 1) * 512],
                            in_=stage[:],
                        )
    nc.finalize()
    return nc


def _get_nc():
    if "nc" not in _CACHE:
        _CACHE["nc"] = _build_nc()
    return _CACHE["nc"]


def make_in_maps(x, Wkqv, bkqv, Wp):
    bf16 = ml_dtypes.bfloat16
    xt = np.ascontiguousarray(np.transpose(x, (0, 2, 1))).astype(bf16)
    pidx = np.arange(P)[:, None]
    fidx = np.arange(512)[None, :]
    masks = np.stack(
        [(pidx + P * i <= fidx) for i in range(4)]
    ).astype(bf16)
    ident = np.eye(P, dtype=bf16)
    ones = np.ones((1, DH), np.float32)
    in_maps = []
    for c in range(NCORES):
        wk = np.ascontiguousarray(Wkqv[HL * c:HL * (c + 1)]).astype(bf16)
        bk = np.asarray(bkqv[HL * c:HL * (c + 1)], np.float32)
        bias = np.zeros((HL, P, ET), np.float32)
        for t in range(16):
            bias[:, :, t] = bk[:, t * P:(t + 1) * P]
        bias[:, :DH, 16] = bk[:, 2 * D:]
        wp = np.ascontiguousarray(Wp[P * c:P * (c + 1)]).astype(bf16)
        in_maps.append({
            "xt": xt, "wkqv": wk, "bias": bias, "wp": wp,
            "masks": masks, "ident": ident, "ones": ones,
        })
    return in_maps


def run(x, Wkqv, bkqv, Wp, bp, trace=False):
    nc = _get_nc()
    in_maps = make_in_maps(x, Wkqv, bkqv, Wp)
    res = run_bass_kernel_spmd(nc, in_maps, core_ids=list(range(NCORES)), trace=trace)
    total = None
    for r in res.results:
        part = r["out"].astype(np.float64)
        total = part if total is None else total + part
    out = (total + np.asarray(bp, np.float64)).astype(np.float32)
    return out, res


def kernel(x, Wkqv, bkqv, Wp, bp):
    out, _ = run(x, Wkqv, bkqv, Wp, bp, trace=False)
    return out


# revision 11
# speedup vs baseline: 1.1760x; 1.1114x over previous
"""Causal self-attention (per-head full-D k/q, DH-wide v) on 8 trn2 cores.

Sharding: tensor-parallel over heads. Core c owns heads (2c, 2c+1).
Each core computes, for all 4 batches:
  y^T[h] = (x @ Wkqv[h] + bkqv[h])^T        (e on partitions, tokens free)
  S^T    = k @ q^T / 32                     (m on partitions, n free)
  P^T    = exp(S^T) * causal_mask           (unnormalized, bf16)
  O^T_aug= [v | 1]^T-stacked @ P^T          (row 64 = softmax denominator)
  O^T    = O^T[0:64] / den                  (per-column normalize)
  partial= [O^T(h0); O^T(h1)].T @ Wp[128c:128c+128]   (f32, DMA'd out)
Host sums the 8 partials and adds bp.

Layouts chosen so no on-device transposes of large tensors are needed:
x is pre-transposed on host; k^T/q^T come out of the projection directly;
P^T is what the A@v matmul wants as moving operand; only v needs an
on-device transpose (8 small 64x128 PE transposes per (b,h)).
"""

import numpy as np
import ml_dtypes

import concourse.bass as bass
import concourse.bacc as bacc
import concourse.tile as tile
from concourse import mybir
from concourse.bass_utils import run_bass_kernel_spmd

BF16 = mybir.dt.bfloat16
F32 = mybir.dt.float32
AF = mybir.ActivationFunctionType

B, N, D, H, DH = 4, 1024, 1024, 16, 64
E = 2 * D + DH          # 2112 = per-head kqv output width
P = 128
NCORES = 8
HL = H // NCORES        # 2 local heads per core
DT = D // P             # 8 contraction tiles
ET = 17                 # e tiles: 8 k + 8 q + 1 v(64 rows)
NB = N // 512           # 2 moving-dim blocks
NT = N // P             # 8 token tiles

_CACHE = {}


def _build_nc():
    nc = bacc.Bacc(
        "TRN2",
        target_bir_lowering=False,
        debug=False,
        enable_asserts=True,
        num_devices=NCORES,
    )
    xt_d = nc.declare_dram_parameter("xt", [B, D, N], BF16, isOutput=False)
    wk_d = nc.declare_dram_parameter("wkqv", [HL, D, E], BF16, isOutput=False)
    bias_d = nc.declare_dram_parameter("bias", [HL, P, ET], F32, isOutput=False)
    wp_d = nc.declare_dram_parameter("wp", [P, D], BF16, isOutput=False)
    mask_d = nc.declare_dram_parameter("masks", [4, P, 512], BF16, isOutput=False)
    id_d = nc.declare_dram_parameter("ident", [P, P], BF16, isOutput=False)
    out_d = nc.declare_dram_parameter("out", [B, N, D], F32, isOutput=True)

    with tile.TileContext(nc) as tc:
        with (
            tc.tile_pool(name="const", bufs=1) as constp,
            tc.tile_pool(name="wpool", bufs=1) as wpool,
            tc.tile_pool(name="xpool", bufs=2) as xpool,
            tc.tile_pool(name="kqpool", bufs=1) as kqpool,
            tc.tile_pool(name="vpool", bufs=2) as vpool,
            tc.tile_pool(name="ptpool", bufs=4) as ptpool,
            tc.tile_pool(name="otpool", bufs=2) as otpool,
            tc.tile_pool(name="stpool", bufs=3) as stpool,
            tc.tile_pool(name="pspool", bufs=1, space="PSUM") as pspool,
        ):
            # ---- resident constants/weights ----
            # DMA issue order matters: the first projection matmuls need
            # wk[h0] + xt[b0] + bias, so those go first; everything else
            # (masks, identity, second head, Wp) is needed later and queues
            # behind them.
            wk_sb = wpool.tile([P, HL * DT * E], BF16, name="wk_sb")
            bias_sb = constp.tile([P, HL * ET], F32, name="bias_sb")
            wp_sb = constp.tile([P, D], BF16, name="wp_sb")
            mask_sb = constp.tile([P, 4 * 512], BF16, name="mask_sb")
            id_sb = constp.tile([P, P], BF16, name="id_sb")

            def dma_wk(h, d):
                nc.sync.dma_start(
                    out=wk_sb[:, (h * DT + d) * E:(h * DT + d + 1) * E],
                    in_=wk_d[h, d * P:(d + 1) * P, :],
                )

            for c4 in range(4):  # split first weight tile so compute starts sooner
                nc.sync.dma_start(
                    out=wk_sb[:, c4 * 528:(c4 + 1) * 528],
                    in_=wk_d[0, 0:P, c4 * 528:(c4 + 1) * 528],
                )
            for h in range(HL):
                nc.sync.dma_start(
                    out=bias_sb[:, h * ET:(h + 1) * ET], in_=bias_d[h]
                )

            for b in range(B):
                xt_sb = xpool.tile([P, DT * N], BF16, tag="xt", name="xt_sb")
                for d in range(DT):
                    if b == 0 and d == 0:
                        for c2 in range(2):
                            nc.sync.dma_start(
                                out=xt_sb[:, c2 * 512:(c2 + 1) * 512],
                                in_=xt_d[0, 0:P, c2 * 512:(c2 + 1) * 512],
                            )
                    else:
                        nc.sync.dma_start(
                            out=xt_sb[:, d * N:(d + 1) * N],
                            in_=xt_d[b, d * P:(d + 1) * P, :],
                        )
                    if b == 0 and d >= 1:
                        dma_wk(0, d)
                if b == 0:
                    nc.sync.dma_start(out=id_sb[:], in_=id_d[:])
                    for m in range(4):
                        nc.sync.dma_start(
                            out=mask_sb[:, m * 512:(m + 1) * 512], in_=mask_d[m]
                        )
                    for d in range(DT):
                        dma_wk(1, d)
                    nc.sync.dma_start(out=wp_sb[:], in_=wp_d[:])
                ostack = otpool.tile([P, N], BF16, tag="ostack", name="ostack")

                for h in range(HL):
                    kt_sb = kqpool.tile([P, DT * N], BF16, tag="kt", name="kt_sb")
                    qt_sb = kqpool.tile([P, DT * N], BF16, tag="qt", name="qt_sb")
                    vt_sb = vpool.tile([DH, N], BF16, tag="vt", name="vt_sb")

                    # ---- kqv projection: y^T[e_tile, n] ----
                    for t in [16] + list(range(16)):
                        rows = P if t < 16 else DH
                        ecol = t * P    # t==16 -> 2048 (v block)
                        for j in range(NB):
                            ps_y = pspool.tile(
                                [P, 512], F32, tag="ps", bufs=3, name="ps_y"
                            )
                            for d in range(DT):
                                wofs = (h * DT + d) * E + ecol
                                nc.tensor.matmul(
                                    ps_y[:rows, :],
                                    lhsT=wk_sb[:, wofs:wofs + rows],
                                    rhs=xt_sb[:, d * N + j * 512: d * N + j * 512 + 512],
                                    start=(d == 0),
                                    stop=(d == DT - 1),
                                )
                            bias_ap = bias_sb[:rows, h * ET + t: h * ET + t + 1]
                            if t < 8:
                                dest = kt_sb[:, t * N + j * 512: t * N + j * 512 + 512]
                            elif t < 16:
                                dest = qt_sb[:, (t - 8) * N + j * 512: (t - 8) * N + j * 512 + 512]
                            else:
                                dest = vt_sb[:, j * 512:(j + 1) * 512]
                            nc.vector.tensor_scalar_add(dest, ps_y[:rows, :], bias_ap)

                    # ---- v: transpose to [m, dh] and append ones column ----
                    v_sb = vpool.tile([P, NT * (DH + 1)], BF16, tag="vaug", name="v_sb")
                    nc.vector.memset(v_sb[:, :], 1.0)
                    for i in range(NT):
                        ps_t = pspool.tile([P, DH], BF16, tag="ps", bufs=3, name="ps_t")
                        nc.tensor.transpose(
                            ps_t[:, :],
                            vt_sb[:, i * P:(i + 1) * P],
                            id_sb[:DH, :DH],
                        )
                        nc.scalar.activation(
                            v_sb[:, i * (DH + 1): i * (DH + 1) + DH], ps_t[:, :], AF.Copy
                        )

                    # ---- attention: S^T tiles, exp, mask, P^T @ [v|1] ----
                    for j in range(NB):
                        nm = 4 * (j + 1)     # causal: valid m tiles for this n block
                        ps_o = pspool.tile([DH + 1, 512], F32, tag="po", bufs=2, name="ps_o")
                        for i in range(nm):
                            ps_s = pspool.tile([P, 512], F32, tag="ps", bufs=3, name="ps_s")
                            for e in range(DT):
                                nc.tensor.matmul(
                                    ps_s[:],
                                    lhsT=kt_sb[:, e * N + i * P: e * N + i * P + P],
                                    rhs=qt_sb[:, e * N + j * 512: e * N + j * 512 + 512],
                                    start=(e == 0),
                                    stop=(e == DT - 1),
                                )
                            pt = ptpool.tile([P, 512], BF16, tag="pt", name="pt")
                            nc.scalar.activation(pt[:], ps_s[:], AF.Exp, scale=1.0 / 32.0)
                            mi = i - 4 * j
                            if mi >= 0:  # partial (diagonal-crossing) tile
                                pt2 = ptpool.tile([P, 512], BF16, tag="pt", name="pt2")
                                nc.vector.tensor_mul(
                                    pt2[:], pt[:], mask_sb[:, mi * 512:(mi + 1) * 512]
                                )
                                pt = pt2
                            nc.tensor.matmul(
                                ps_o[:],
                                lhsT=v_sb[:, i * (DH + 1):(i + 1) * (DH + 1)],
                                rhs=pt[:],
                                start=(i == 0),
                                stop=(i == nm - 1),
                            )
                        # ---- normalize by denominator (row 64 of ps_o) ----
                        ot = otpool.tile([DH, 512], F32, tag="ot", name="ot")
                        nc.scalar.activation(ot[:], ps_o[:DH, :], AF.Copy)
                        den_row = otpool.tile([1, 512], F32, tag="den", name="den_row")
                        nc.scalar.activation(den_row[:], ps_o[DH:DH + 1, :], AF.Copy)
                        den_b = stpool.tile([DH, 512], F32, tag="denb", name="den_b")
                        nc.gpsimd.partition_broadcast(den_b[:], den_row[:], channels=DH)
                        recip = stpool.tile([DH, 512], F32, tag="recip", name="recip")
                        nc.vector.reciprocal(recip[:], den_b[:])
                        nc.vector.tensor_mul(
                            ostack[h * DH:(h + 1) * DH, j * 512:(j + 1) * 512],
                            ot[:], recip[:],
                        )

                # ---- output projection: both heads at once (K=128) ----
                for t in range(NT):
                    for j2 in range(NB):
                        ps_f = pspool.tile([P, 512], F32, tag="psf", bufs=2, name="ps_f")
                        nc.tensor.matmul(
                            ps_f[:],
                            lhsT=ostack[:, t * P:(t + 1) * P],
                            rhs=wp_sb[:, j2 * 512:(j2 + 1) * 512],
                            start=True, stop=True,
                        )
                        stage = stpool.tile([P, 512], F32, tag="stage", name="stage")
                        if (t * NB + j2) % 2 == 0:
                            nc.scalar.activation(stage[:], ps_f[:], AF.Copy)
                        else:
                            nc.vector.tensor_copy(stage[:], ps_f[:])
                        nc.sync.dma_start(
                            out=out_d[b, t * P:(t + 1) * P, j2 * 512:(j2 + 1) * 512],
                            in_=stage[:],
                        )
    nc.finalize()
    return nc


def _get_nc():
    if "nc" not in _CACHE:
        _CACHE["nc"] = _build_nc()
    return _CACHE["nc"]


def make_in_maps(x, Wkqv, bkqv, Wp):
    bf16 = ml_dtypes.bfloat16
    xt = np.ascontiguousarray(np.transpose(x, (0, 2, 1))).astype(bf16)
    pidx = np.arange(P)[:, None]
    fidx = np.arange(512)[None, :]
    masks = np.stack(
        [(pidx + P * i <= fidx) for i in range(4)]
    ).astype(bf16)
    ident = np.eye(P, dtype=bf16)
    ones = np.ones((1, DH), np.float32)
    in_maps = []
    for c in range(NCORES):
        wk = np.ascontiguousarray(Wkqv[HL * c:HL * (c + 1)]).astype(bf16)
        bk = np.asarray(bkqv[HL * c:HL * (c + 1)], np.float32)
        bias = np.zeros((HL, P, ET), np.float32)
        for t in range(16):
            bias[:, :, t] = bk[:, t * P:(t + 1) * P]
        bias[:, :DH, 16] = bk[:, 2 * D:]
        wp = np.ascontiguousarray(Wp[P * c:P * (c + 1)]).astype(bf16)
        in_maps.append({
            "xt": xt, "wkqv": wk, "bias": bias, "wp": wp,
            "masks": masks, "ident": ident, "ones": ones,
        })
    return in_maps


def run(x, Wkqv, bkqv, Wp, bp, trace=False):
    nc = _get_nc()
    in_maps = make_in_maps(x, Wkqv, bkqv, Wp)
    res = run_bass_kernel_spmd(nc, in_maps, core_ids=list(range(NCORES)), trace=trace)
    total = None
    for r in res.results:
        part = r["out"].astype(np.float64)
        total = part if total is None else total + part
    out = (total + np.asarray(bp, np.float64)).astype(np.float32)
    return out, res


def kernel(x, Wkqv, bkqv, Wp, bp):
    out, _ = run(x, Wkqv, bkqv, Wp, bp, trace=False)
    return out


# revision 14
# speedup vs baseline: 1.2032x; 1.0231x over previous
"""Causal self-attention (per-head full-D k/q, DH-wide v) on 8 trn2 cores.

Sharding: tensor-parallel over heads. Core c owns heads (2c, 2c+1).
Each core computes, for all 4 batches:
  y^T[h] = (x @ Wkqv[h] + bkqv[h])^T        (e on partitions, tokens free)
  S^T    = k @ q^T / 32                     (m on partitions, n free)
  P^T    = exp(S^T) * causal_mask           (unnormalized, bf16)
  O^T_aug= [v | 1]^T-stacked @ P^T          (row 64 = softmax denominator)
  O^T    = O^T[0:64] / den                  (per-column normalize)
  partial= [O^T(h0); O^T(h1)].T @ Wp[128c:128c+128]   (f32, DMA'd out)
Host sums the 8 partials and adds bp.

Layouts chosen so no on-device transposes of large tensors are needed:
x is pre-transposed on host; k^T/q^T come out of the projection directly;
P^T is what the A@v matmul wants as moving operand; only v needs an
on-device transpose (8 small 64x128 PE transposes per (b,h)).
"""

import numpy as np
import ml_dtypes

import concourse.bass as bass
import concourse.bacc as bacc
import concourse.tile as tile
from concourse import mybir
from concourse.bass_utils import run_bass_kernel_spmd

BF16 = mybir.dt.bfloat16
F32 = mybir.dt.float32
AF = mybir.ActivationFunctionType

B, N, D, H, DH = 4, 1024, 1024, 16, 64
E = 2 * D + DH          # 2112 = per-head kqv output width
P = 128
NCORES = 8
HL = H // NCORES        # 2 local heads per core
DT = D // P             # 8 contraction tiles
ET = 17                 # e tiles: 8 k + 8 q + 1 v(64 rows)
NB = N // 512           # 2 moving-dim blocks
NT = N // P             # 8 token tiles

_CACHE = {}


def _build_nc():
    nc = bacc.Bacc(
        "TRN2",
        target_bir_lowering=False,
        debug=False,
        enable_asserts=True,
        num_devices=NCORES,
    )
    xt_d = nc.declare_dram_parameter("xt", [B, D, N], BF16, isOutput=False)
    wk_d = nc.declare_dram_parameter("wkqv", [HL, D, E], BF16, isOutput=False)
    bias_d = nc.declare_dram_parameter("bias", [HL, P, ET], F32, isOutput=False)
    wp_d = nc.declare_dram_parameter("wp", [P, D], BF16, isOutput=False)
    mask_d = nc.declare_dram_parameter("masks", [2, P, 256], BF16, isOutput=False)
    id_d = nc.declare_dram_parameter("ident", [P, P], BF16, isOutput=False)
    out_d = nc.declare_dram_parameter("out", [B, N, D], F32, isOutput=True)

    with tile.TileContext(nc) as tc:
        with (
            tc.tile_pool(name="const", bufs=1) as constp,
            tc.tile_pool(name="wpool", bufs=1) as wpool,
            tc.tile_pool(name="xpool", bufs=2) as xpool,
            tc.tile_pool(name="kqpool", bufs=1) as kqpool,
            tc.tile_pool(name="vpool", bufs=2) as vpool,
            tc.tile_pool(name="ptpool", bufs=4) as ptpool,
            tc.tile_pool(name="otpool", bufs=2) as otpool,
            tc.tile_pool(name="stpool", bufs=3) as stpool,
            tc.tile_pool(name="pspool", bufs=1, space="PSUM") as pspool,
        ):
            # ---- resident constants/weights ----
            # DMA issue order matters: the first projection matmuls need
            # wk[h0] + xt[b0] + bias, so those go first; everything else
            # (masks, identity, second head, Wp) is needed later and queues
            # behind them.
            wk_sb = wpool.tile([P, HL * DT * E], BF16, name="wk_sb")
            bias_sb = constp.tile([P, HL * ET], F32, name="bias_sb")
            wp_sb = constp.tile([P, D], BF16, name="wp_sb")
            mask_sb = constp.tile([P, 2 * 256], BF16, name="mask_sb")
            id_sb = constp.tile([P, P], BF16, name="id_sb")

            def dma_wk(h, d):
                nc.sync.dma_start(
                    out=wk_sb[:, (h * DT + d) * E:(h * DT + d + 1) * E],
                    in_=wk_d[h, d * P:(d + 1) * P, :],
                )

            for c4 in range(4):  # split first weight tile so compute starts sooner
                nc.sync.dma_start(
                    out=wk_sb[:, c4 * 528:(c4 + 1) * 528],
                    in_=wk_d[0, 0:P, c4 * 528:(c4 + 1) * 528],
                )
            for h in range(HL):
                nc.sync.dma_start(
                    out=bias_sb[:, h * ET:(h + 1) * ET], in_=bias_d[h]
                )

            for b in range(B):
                xt_sb = xpool.tile([P, DT * N], BF16, tag="xt", name="xt_sb")
                for d in range(DT):
                    if b == 0 and d == 0:
                        for c2 in range(2):
                            nc.sync.dma_start(
                                out=xt_sb[:, c2 * 512:(c2 + 1) * 512],
                                in_=xt_d[0, 0:P, c2 * 512:(c2 + 1) * 512],
                            )
                    else:
                        nc.sync.dma_start(
                            out=xt_sb[:, d * N:(d + 1) * N],
                            in_=xt_d[b, d * P:(d + 1) * P, :],
                        )
                    if b == 0 and d >= 1:
                        dma_wk(0, d)
                if b == 0:
                    nc.sync.dma_start(out=id_sb[:], in_=id_d[:])
                    for m in range(2):
                        nc.sync.dma_start(
                            out=mask_sb[:, m * 256:(m + 1) * 256], in_=mask_d[m]
                        )
                    for d in range(DT):
                        dma_wk(1, d)
                    nc.sync.dma_start(out=wp_sb[:], in_=wp_d[:])
                ostack = otpool.tile([P, N], BF16, tag="ostack", name="ostack")

                for h in range(HL):
                    kt_sb = kqpool.tile([P, DT * N], BF16, tag="kt", name="kt_sb")
                    qt_sb = kqpool.tile([P, DT * N], BF16, tag="qt", name="qt_sb")
                    vt_sb = vpool.tile([DH, N], BF16, tag="vt", name="vt_sb")

                    # ---- kqv projection: y^T[e_tile, n] ----
                    for t in [16] + list(range(16)):
                        rows = P if t < 16 else DH
                        ecol = t * P    # t==16 -> 2048 (v block)
                        for j in range(NB):
                            ps_y = pspool.tile(
                                [P, 512], F32, tag="ps", bufs=3, name="ps_y"
                            )
                            for d in range(DT):
                                wofs = (h * DT + d) * E + ecol
                                nc.tensor.matmul(
                                    ps_y[:rows, :],
                                    lhsT=wk_sb[:, wofs:wofs + rows],
                                    rhs=xt_sb[:, d * N + j * 512: d * N + j * 512 + 512],
                                    start=(d == 0),
                                    stop=(d == DT - 1),
                                )
                            bias_ap = bias_sb[:rows, h * ET + t: h * ET + t + 1]
                            if t < 8:
                                dest = kt_sb[:, t * N + j * 512: t * N + j * 512 + 512]
                            elif t < 16:
                                dest = qt_sb[:, (t - 8) * N + j * 512: (t - 8) * N + j * 512 + 512]
                            else:
                                dest = vt_sb[:, j * 512:(j + 1) * 512]
                            nc.vector.tensor_scalar_add(dest, ps_y[:rows, :], bias_ap)

                    # ---- v: transpose to [m, dh] and append ones column ----
                    v_sb = vpool.tile([P, NT * (DH + 1)], BF16, tag="vaug", name="v_sb")
                    nc.vector.memset(v_sb[:, :], 1.0)
                    for i in range(NT):
                        ps_t = pspool.tile([P, DH], BF16, tag="ps", bufs=3, name="ps_t")
                        nc.tensor.transpose(
                            ps_t[:, :],
                            vt_sb[:, i * P:(i + 1) * P],
                            id_sb[:DH, :DH],
                        )
                        nc.scalar.activation(
                            v_sb[:, i * (DH + 1): i * (DH + 1) + DH], ps_t[:, :], AF.Copy
                        )

                    # ---- attention: S^T tiles (256-wide n blocks for finer
                    # causal skipping), exp, mask, P^T @ [v|1] ----
                    for j in range(4):
                        nm = 2 * j + 2   # causal: valid m tiles for this n block
                        ps_o = pspool.tile([DH + 1, 256], F32, tag="po", bufs=2, name="ps_o")
                        for i in range(nm):
                            ps_s = pspool.tile([P, 256], F32, tag="ps", bufs=3, name="ps_s")
                            for e in range(DT):
                                nc.tensor.matmul(
                                    ps_s[:],
                                    lhsT=kt_sb[:, e * N + i * P: e * N + i * P + P],
                                    rhs=qt_sb[:, e * N + j * 256: e * N + j * 256 + 256],
                                    start=(e == 0),
                                    stop=(e == DT - 1),
                                )
                            pt = ptpool.tile([P, 256], BF16, tag="pt", name="pt")
                            nc.scalar.activation(pt[:], ps_s[:], AF.Exp, scale=1.0 / 32.0)
                            mi = i - 2 * j
                            if mi >= 0:  # partial (diagonal-crossing) tile
                                pt2 = ptpool.tile([P, 256], BF16, tag="pt", name="pt2")
                                nc.vector.tensor_mul(
                                    pt2[:], pt[:], mask_sb[:, mi * 256:(mi + 1) * 256]
                                )
                                pt = pt2
                            nc.tensor.matmul(
                                ps_o[:],
                                lhsT=v_sb[:, i * (DH + 1):(i + 1) * (DH + 1)],
                                rhs=pt[:],
                                start=(i == 0),
                                stop=(i == nm - 1),
                            )
                        # ---- normalize by denominator (row 64 of ps_o) ----
                        ot = otpool.tile([DH, 256], F32, tag="ot", name="ot")
                        nc.scalar.activation(ot[:], ps_o[:DH, :], AF.Copy)
                        den_row = otpool.tile([1, 256], F32, tag="den", name="den_row")
                        nc.scalar.activation(den_row[:], ps_o[DH:DH + 1, :], AF.Copy)
                        den_b = stpool.tile([DH, 256], F32, tag="denb", name="den_b")
                        nc.gpsimd.partition_broadcast(den_b[:], den_row[:], channels=DH)
                        recip = stpool.tile([DH, 256], F32, tag="recip", name="recip")
                        nc.vector.reciprocal(recip[:], den_b[:])
                        nc.vector.tensor_mul(
                            ostack[h * DH:(h + 1) * DH, j * 256:(j + 1) * 256],
                            ot[:], recip[:],
                        )
                        # ---- output projection, interleaved: once head 1 has
                        # normalized n block j, token tiles 2j, 2j+1 are ready
                        # (both heads done for those columns) ----
                        if h == HL - 1:
                            for t in (2 * j, 2 * j + 1):
                                for j2 in range(NB):
                                    ps_f = pspool.tile([P, 512], F32, tag="psf", bufs=2, name="ps_f")
                                    nc.tensor.matmul(
                                        ps_f[:],
                                        lhsT=ostack[:, t * P:(t + 1) * P],
                                        rhs=wp_sb[:, j2 * 512:(j2 + 1) * 512],
                                        start=True, stop=True,
                                    )
                                    stage = stpool.tile([P, 512], F32, tag="stage", name="stage")
                                    if (t * NB + j2) % 2 == 0:
                                        nc.scalar.activation(stage[:], ps_f[:], AF.Copy)
                                    else:
                                        nc.vector.tensor_copy(stage[:], ps_f[:])
                                    nc.sync.dma_start(
                                        out=out_d[b, t * P:(t + 1) * P, j2 * 512:(j2 + 1) * 512],
                                        in_=stage[:],
                                    )

    nc.finalize()
    return nc


def _get_nc():
    if "nc" not in _CACHE:
        _CACHE["nc"] = _build_nc()
    return _CACHE["nc"]


def make_in_maps(x, Wkqv, bkqv, Wp):
    bf16 = ml_dtypes.bfloat16
    xt = np.ascontiguousarray(np.transpose(x, (0, 2, 1))).astype(bf16)
    pidx = np.arange(P)[:, None]
    fidx = np.arange(256)[None, :]
    masks = np.stack(
        [(pidx + P * i <= fidx) for i in range(2)]
    ).astype(bf16)
    ident = np.eye(P, dtype=bf16)
    in_maps = []
    for c in range(NCORES):
        wk = np.ascontiguousarray(Wkqv[HL * c:HL * (c + 1)]).astype(bf16)
        bk = np.asarray(bkqv[HL * c:HL * (c + 1)], np.float32)
        bias = np.zeros((HL, P, ET), np.float32)
        for t in range(16):
            bias[:, :, t] = bk[:, t * P:(t + 1) * P]
        bias[:, :DH, 16] = bk[:, 2 * D:]
        wp = np.ascontiguousarray(Wp[P * c:P * (c + 1)]).astype(bf16)
        in_maps.append({
            "xt": xt, "wkqv": wk, "bias": bias, "wp": wp,
            "masks": masks, "ident": ident,
        })
    return in_maps


def run(x, Wkqv, bkqv, Wp, bp, trace=False):
    nc = _get_nc()
    in_maps = make_in_maps(x, Wkqv, bkqv, Wp)
    res = run_bass_kernel_spmd(nc, in_maps, core_ids=list(range(NCORES)), trace=trace)
    total = None
    for r in res.results:
        part = r["out"].astype(np.float64)
        total = part if total is None else total + part
    out = (total + np.asarray(bp, np.float64)).astype(np.float32)
    return out, res


def kernel(x, Wkqv, bkqv, Wp, bp):
    out, _ = run(x, Wkqv, bkqv, Wp, bp, trace=False)
    return out
